# revision 3
# baseline (speedup 1.0000x reference)
"""Binary-weight 3x3 conv (stride 1, pad 1) on 8 TRN2 NeuronCores.

Strategy: data-parallel over batch (4 images per core), weights replicated.
Per image the conv is 9 shifted [Cin,Cout] matmuls accumulated in PSUM:
  out[co, h, w] = sum_{cit,kh,kw} Wb[kh,kw][ci,co].T @ xpad[ci, h+kh, w+kw]
with channels on the partition dim (NCHW layout already has x[n] as a
[C, H*W] channel-major matrix). Input is zero-padded to [128, 58, 58] in
SBUF; matmul rhs uses a 3D access pattern [128, 8 rows, 56 cols] so each
matmul covers 8 output rows (N=448). fp32r (reduced-precision fp32 matmul
mode) runs at 1 cycle/row on the PE, 4x faster than plain fp32; the
binarized weights (+/-1, 0) are exact in any dtype.
"""

import numpy as np

N_CORES = 8
B_PER_CORE = 4  # 32 images / 8 cores
CIN = 256
COUT = 256
H = W = 56
HP = WP = 58  # padded
RB = 8  # output rows per matmul
NBLK = H // RB  # 7
NFREE = RB * W  # 448

_CACHED = {}


def _build_nc():
    import concourse.mybir as mybir
    from concourse import bacc
    from concourse.tile import TileContext

    f32 = mybir.dt.float32
    f32r = mybir.dt.float32r

    nc = bacc.Bacc("TRN2", target_bir_lowering=False, debug=False)
    xs = nc.dram_tensor("xs", [B_PER_CORE, CIN, H, W], f32, kind="ExternalInput").ap()
    wt = nc.dram_tensor("wt", [4, 128, 9, 128], f32, kind="ExternalInput").ap()
    out = nc.dram_tensor(
        "out", [B_PER_CORE, COUT, H, W], f32, kind="ExternalOutput"
    ).ap()

    with TileContext(nc) as tc:
        with (
            tc.tile_pool(name="wp", bufs=1) as wp,
            tc.tile_pool(name="xp", bufs=8) as xp,
            tc.tile_pool(name="yp", bufs=6) as yp,
            tc.tile_pool(name="pp", bufs=4, space="PSUM") as pp,
        ):
            w_sb = wp.tile([128, 4, 9, 128], f32r, name="w_sb")
            for g in range(4):
                nc.sync.dma_start(out=w_sb[:, g], in_=wt[g].bitcast(f32r))

            # fp32r memset is not a legal ISA op; round zeros in via DVE copy
            zrow = wp.tile([128, HP], f32, name="zrow")
            nc.vector.memset(zrow[:], 0.0)

            xt = {}
            for n in range(B_PER_CORE):
                for cit in range(2):
                    t = xp.tile([128, HP, WP], f32r, name=f"xpad_{n}_{cit}", tag="xpad")
                    nc.vector.tensor_copy(out=t[:, 0, :], in_=zrow[:])
                    nc.vector.tensor_copy(out=t[:, HP - 1, :], in_=zrow[:])
                    nc.vector.tensor_copy(out=t[:, 1 : HP - 1, 0:1], in_=zrow[:, : HP - 2])
                    nc.vector.tensor_copy(
                        out=t[:, 1 : HP - 1, WP - 1 : WP], in_=zrow[:, : HP - 2]
                    )
                    nc.sync.dma_start(
                        out=t[:, 1 : HP - 1, 1 : WP - 1],
                        in_=xs[n, cit * 128 : (cit + 1) * 128].bitcast(f32r),
                    )
                    xt[(n, cit)] = t

            for n in range(B_PER_CORE):
                for ct in range(2):
                    for blk in range(NBLK):
                        ps = pp.tile([128, NFREE], f32, name="ps", tag="ps")
                        h0 = blk * RB
                        idx = 0
                        for cit in range(2):
                            for k in range(9):
                                kh, kw = divmod(k, 3)
                                rhs = xt[(n, cit)][
                                    :, h0 + kh : h0 + kh + RB, kw : kw + W
                                ]
                                lhsT = w_sb[:, ct * 2 + cit, k, :]
                                nc.tensor.matmul(
                                    ps[:],
                                    lhsT=lhsT,
                                    rhs=rhs,
                                    start=(idx == 0),
                                    stop=(idx == 17),
                                )
                                idx += 1
                        y = yp.tile([128, NFREE], f32, name="y", tag="y")
                        nc.vector.tensor_copy(out=y[:], in_=ps[:])
                        nc.sync.dma_start(
                            out=out[n, ct * 128 : (ct + 1) * 128, h0 : h0 + RB, :],
                            in_=y[:],
                        )
    nc.compile()
    return nc


def _get_nc():
    if "nc" not in _CACHED:
        _CACHED["nc"] = _build_nc()
    return _CACHED["nc"]


def _prep_weights(W_arr):
    Wb = np.sign(np.asarray(W_arr, dtype=np.float32))
    # [co, ci, kh, kw] -> [ct, cit, ci, k, co] -> [4, 128, 9, 128]
    wt = (
        Wb.reshape(2, 128, 2, 128, 9)
        .transpose(0, 2, 3, 4, 1)
        .reshape(4, 128, 9, 128)
    )
    return np.ascontiguousarray(wt)


def run(x, W, trace=False, trace_kwargs=None):
    from concourse.bass_utils import run_bass_kernel_spmd

    x = np.asarray(x, dtype=np.float32)
    wt = _prep_weights(W)
    nc = _get_nc()
    in_maps = [
        {"xs": np.ascontiguousarray(x[i * B_PER_CORE : (i + 1) * B_PER_CORE]), "wt": wt}
        for i in range(N_CORES)
    ]
    res = run_bass_kernel_spmd(
        nc,
        in_maps,
        list(range(N_CORES)),
        trace=trace,
        trace_kwargs=trace_kwargs or {},
    )
    out = np.concatenate([np.asarray(res.results[i]["out"]) for i in range(N_CORES)])
    return out, res


def kernel(x, W):
    out, _ = run(x, W, trace=False)
    return out


# revision 4
# speedup vs baseline: 1.0816x; 1.0816x over previous
"""Binary-weight 3x3 conv (stride 1, pad 1) on 8 TRN2 NeuronCores.

Strategy: data-parallel over batch (4 images per core), weights replicated.
Per image the conv is 9 shifted [Cin,Cout] matmuls accumulated in PSUM:
  out[co, h, w] = sum_{cit,kh,kw} Wb[kh,kw][ci,co].T @ xpad[ci, h+kh, w+kw]
with channels on the partition dim (NCHW layout already has x[n] as a
[C, H*W] channel-major matrix). The input is DMA'd contiguously into a
staging tile (fast: 12.5KB runs/partition), then padded to [128, 58, 58]
fp32r in SBUF by a DVE copy (which doubles as the fp32r rounding op the
BIR verifier requires). Matmul rhs uses a 3D access pattern
[128, 8 rows, 56 cols] so each matmul covers 8 output rows (N=448).
fp32r (reduced-precision fp32 matmul mode) runs at 1 cycle/row on the
PE, 4x faster than plain fp32; binarized weights (+/-1, 0) are exact.
"""

import numpy as np

N_CORES = 8
B_PER_CORE = 4  # 32 images / 8 cores
CIN = 256
COUT = 256
H = W = 56
HP = WP = 58  # padded
RB = 8  # output rows per matmul
NBLK = H // RB  # 7
NFREE = RB * W  # 448

_CACHED = {}


def _build_nc():
    import concourse.mybir as mybir
    from concourse import bacc
    from concourse.tile import TileContext

    f32 = mybir.dt.float32
    f32r = mybir.dt.float32r

    nc = bacc.Bacc("TRN2", target_bir_lowering=False, debug=False)
    xs = nc.dram_tensor("xs", [B_PER_CORE, CIN, H, W], f32, kind="ExternalInput").ap()
    wt = nc.dram_tensor("wt", [4, 128, 9, 128], f32, kind="ExternalInput").ap()
    out = nc.dram_tensor(
        "out", [B_PER_CORE, COUT, H, W], f32, kind="ExternalOutput"
    ).ap()

    with TileContext(nc) as tc:
        with (
            tc.tile_pool(name="wp", bufs=1) as wp,
            tc.tile_pool(name="sp", bufs=2) as sp,
            tc.tile_pool(name="xp", bufs=8) as xp,
            tc.tile_pool(name="yp", bufs=6) as yp,
            tc.tile_pool(name="pp", bufs=4, space="PSUM") as pp,
        ):
            w_sb = wp.tile([128, 4, 9, 128], f32r, name="w_sb")
            for g in range(4):
                nc.sync.dma_start(out=w_sb[:, g], in_=wt[g].bitcast(f32r))

            # fp32r memset is not a legal ISA op; round zeros in via DVE copy
            zrow = wp.tile([128, HP], f32, name="zrow")
            nc.vector.memset(zrow[:], 0.0)

            xt = {}
            for n in range(B_PER_CORE):
                for cit in range(2):
                    stage = sp.tile([128, H, W], f32, name="stage", tag="stage")
                    nc.sync.dma_start(
                        out=stage[:], in_=xs[n, cit * 128 : (cit + 1) * 128]
                    )
                    t = xp.tile([128, HP, WP], f32r, name=f"xpad_{n}_{cit}", tag="xpad")
                    nc.vector.tensor_copy(out=t[:, 0, :], in_=zrow[:])
                    nc.vector.tensor_copy(out=t[:, HP - 1, :], in_=zrow[:])
                    nc.vector.tensor_copy(
                        out=t[:, 1 : HP - 1, 0:1], in_=zrow[:, : HP - 2]
                    )
                    nc.vector.tensor_copy(
                        out=t[:, 1 : HP - 1, WP - 1 : WP], in_=zrow[:, : HP - 2]
                    )
                    nc.vector.tensor_copy(
                        out=t[:, 1 : HP - 1, 1 : WP - 1], in_=stage[:]
                    )
                    xt[(n, cit)] = t

            for n in range(B_PER_CORE):
                for ct in range(2):
                    for blk in range(NBLK):
                        ps = pp.tile([128, NFREE], f32, name="ps", tag="ps")
                        h0 = blk * RB
                        idx = 0
                        for cit in range(2):
                            for k in range(9):
                                kh, kw = divmod(k, 3)
                                rhs = xt[(n, cit)][
                                    :, h0 + kh : h0 + kh + RB, kw : kw + W
                                ]
                                lhsT = w_sb[:, ct * 2 + cit, k, :]
                                nc.tensor.matmul(
                                    ps[:],
                                    lhsT=lhsT,
                                    rhs=rhs,
                                    start=(idx == 0),
                                    stop=(idx == 17),
                                )
                                idx += 1
                        y = yp.tile([128, NFREE], f32, name="y", tag="y")
                        nc.vector.tensor_copy(out=y[:], in_=ps[:])
                        nc.sync.dma_start(
                            out=out[n, ct * 128 : (ct + 1) * 128, h0 : h0 + RB, :],
                            in_=y[:],
                        )
    nc.compile()
    return nc


def _get_nc():
    if "nc" not in _CACHED:
        _CACHED["nc"] = _build_nc()
    return _CACHED["nc"]


def _prep_weights(W_arr):
    Wb = np.sign(np.asarray(W_arr, dtype=np.float32))
    # [co, ci, kh, kw] -> [ct, cit, ci, k, co] -> [4, 128, 9, 128]
    wt = (
        Wb.reshape(2, 128, 2, 128, 9)
        .transpose(0, 2, 3, 4, 1)
        .reshape(4, 128, 9, 128)
    )
    return np.ascontiguousarray(wt)


def run(x, W, trace=False, trace_kwargs=None):
    from concourse.bass_utils import run_bass_kernel_spmd

    x = np.asarray(x, dtype=np.float32)
    wt = _prep_weights(W)
    nc = _get_nc()
    in_maps = [
        {"xs": np.ascontiguousarray(x[i * B_PER_CORE : (i + 1) * B_PER_CORE]), "wt": wt}
        for i in range(N_CORES)
    ]
    res = run_bass_kernel_spmd(
        nc,
        in_maps,
        list(range(N_CORES)),
        trace=trace,
        trace_kwargs=trace_kwargs or {},
    )
    out = np.concatenate([np.asarray(res.results[i]["out"]) for i in range(N_CORES)])
    return out, res


def kernel(x, W):
    out, _ = run(x, W, trace=False)
    return out
